# revision 13
# baseline (speedup 1.0000x reference)
"""MoE router kernel (GPT-OSS TopK router) for 8 Trainium2 NeuronCores.

Computation (per reference):
    logits = hidden_states @ weight.T + bias        # [T=16384, E=128]
    top_v, top_i = top_k(logits, 4)                 # [T, 4]
    top_v = softmax(top_v, axis=-1)
    return (top_v, top_i, logits)

Sharding: token dim split 8 ways (2048 tokens/core); weight+bias replicated.
The x shard and weight are fed to the device pre-transposed and pre-tiled
into the exact per-group chunk order the kernel consumes, so every DMA
reads DRAM sequentially with 2KB descriptors. The bias is folded in by
appending a ones-row to xT and a bias-row to wT (H -> 2881 rows), so
logits = xT_pad.T @ wT_pad exactly (fp32 — float32r flips top-k indices).

Device pipeline per core, per 512-token group:
  4 sub-DMAs load the group's x chunks; 23-chunk fp32 matmul accumulation
  with the WEIGHT stationary into PSUM logitsT [128 exp, 512 tok]; ACT
  copy to SBUF; PE-transpose each 128-token tile back to [tok, exp]; ACT
  copy; DMA logits out; DVE max8/find_index8 for top-4 values+indices;
  ACT exp (bias=-max, accumulated sum); DVE reciprocal + scale. Scalar
  outputs are staged in SBUF and written once at the end.
"""

import sys

for _p in ("/opt/trn_rl_repo",):
    if _p not in sys.path:
        sys.path.insert(0, _p)

import numpy as np

import concourse.bass as bass
import concourse.mybir as mybir
from concourse import bacc
from concourse.bass_utils import run_bass_kernel_spmd
from concourse.masks import make_identity
from concourse.tile import TileContext

# Problem shape (hardcoded per contract)
T = 16384
H = 2880
E = 128
K = 4
N_CORES = 8
TC = T // N_CORES  # tokens per core = 2048

HP = H + 1  # padded contraction dim: ones/bias row folded in
HC = H // 128  # full 128-row contraction chunks = 22
HREM = HP - HC * 128  # 65 (64 x-rows + ones row)
# chunk ranges per x sub-DMA (earlier matmul start + finer DMA pipelining)
XSPLITS = [(0, 6), (6, 12), (12, 17), (17, 22)]

TG = 512  # tokens per group (PSUM bank free-dim)
N_GROUPS = TC // TG  # 4
SUBT = TG // 128  # 128-token subtiles per group = 4
N_TILES = TC // 128  # 16 token tiles per core

FP32 = mybir.dt.float32
I32 = mybir.dt.int32
U32 = mybir.dt.uint32


def _build_program():
    nc = bacc.Bacc("TRN2", target_bir_lowering=False, debug=False)

    # x main chunks, pre-tiled on host: [group, chunk, partition, token]
    xm = nc.declare_dram_parameter(
        "xm", [N_GROUPS, HC, 128, TG], FP32, isOutput=False
    ).ap()
    # x remainder rows (incl. ones row): [group, row, token]
    xr_d = nc.declare_dram_parameter(
        "xr", [N_GROUPS, HREM, TG], FP32, isOutput=False
    ).ap()
    wT = nc.declare_dram_parameter("wT", [HP, E], FP32, isOutput=False).ap()
    out_logits = nc.declare_dram_parameter(
        "out_logits", [TC, E], FP32, isOutput=True
    ).ap()
    out_vals = nc.declare_dram_parameter("out_vals", [TC, K], FP32, isOutput=True).ap()
    out_idx = nc.declare_dram_parameter("out_idx", [TC, K], I32, isOutput=True).ap()

    with TileContext(nc) as tc:
        with (
            tc.tile_pool(name="wpool", bufs=1) as wpool,
            tc.tile_pool(name="xpool", bufs=2) as xpool,
            tc.tile_pool(name="xrem", bufs=2) as xrempool,
            tc.tile_pool(name="psg", bufs=2, space="PSUM") as psg,
            tc.tile_pool(name="pst", bufs=4, space="PSUM") as pst,
            tc.tile_pool(name="ltpool", bufs=2) as ltpool,
            tc.tile_pool(name="lpool", bufs=3) as lpool,
            tc.tile_pool(name="small", bufs=4) as small,
        ):
            # One-time loads: transposed weight(+bias row), identity.
            # W is split so the first matmul only waits on chunks 0-5.
            WSPLIT = 6
            wt_main = wpool.tile([128, HC, E], FP32)
            nc.sync.dma_start(
                wt_main[:, :WSPLIT, :],
                wT[: WSPLIT * 128, :].rearrange("(c p) e -> p c e", p=128),
            )
            nc.sync.dma_start(
                wt_main[:, WSPLIT:, :],
                wT[WSPLIT * 128 : HC * 128, :].rearrange("(c p) e -> p c e", p=128),
            )
            wt_rem = wpool.tile([HREM, E], FP32)
            nc.sync.dma_start(wt_rem, wT[HC * 128 :, :])
            identity = wpool.tile([128, 128], FP32)
            make_identity(nc, identity)

            for g in range(N_GROUPS):
                xs = []
                for si, (c0, c1) in enumerate(XSPLITS):
                    xt = xpool.tile([128, c1 - c0, TG], FP32, tag=f"xs{si}")
                    nc.sync.dma_start(xt, xm[g, c0:c1].rearrange("c p t -> p c t"))
                    xs.append(xt)
                xr = xrempool.tile([HREM, TG], FP32)
                nc.sync.dma_start(xr, xr_d[g])

                # logitsT[e, t] accumulated over 23 chunks; weight stationary
                ps = psg.tile([128, TG], FP32)
                for si, (c0, c1) in enumerate(XSPLITS):
                    for c in range(c0, c1):
                        nc.tensor.matmul(
                            ps,
                            lhsT=wt_main[:, c, :],
                            rhs=xs[si][:, c - c0, :],
                            start=(c == 0),
                            stop=False,
                        )
                nc.tensor.matmul(ps, lhsT=wt_rem, rhs=xr, start=False, stop=True)

                ltT = ltpool.tile([128, TG], FP32)
                nc.scalar.copy(ltT, ps)

                for s in range(SUBT):
                    j = g * SUBT + s  # token tile index (0..15)
                    tok = j * 128

                    ps_t = pst.tile([128, 128], FP32)
                    nc.tensor.transpose(
                        ps_t, ltT[:, s * 128 : (s + 1) * 128], identity
                    )
                    logits_sb = lpool.tile([128, E], FP32)
                    nc.scalar.copy(logits_sb, ps_t)
                    nc.sync.dma_start(out_logits[tok : tok + 128, :], logits_sb)

                    top8v = small.tile([128, 8], FP32)
                    nc.vector.max(top8v, logits_sb)
                    top8i = small.tile([128, 8], U32)
                    nc.vector.max_index(top8i, top8v, logits_sb)

                    negmax = small.tile([128, 1], FP32)
                    nc.vector.tensor_scalar_mul(negmax, top8v[:, 0:1], -1.0)
                    expv = small.tile([128, K], FP32)
                    sum4 = small.tile([128, 1], FP32)
                    nc.scalar.activation(
                        expv,
                        top8v[:, 0:K],
                        mybir.ActivationFunctionType.Exp,
                        bias=negmax,
                        scale=1.0,
                        accum_out=sum4,
                    )
                    rsum = small.tile([128, 1], FP32)
                    nc.vector.reciprocal(rsum, sum4)
                    vals4 = small.tile([128, K], FP32)
                    nc.vector.tensor_scalar_mul(vals4, expv, rsum)
                    idx4 = small.tile([128, K], I32)
                    nc.vector.tensor_copy(idx4, top8i[:, 0:K])
                    nc.sync.dma_start(out_vals[tok : tok + 128, :], vals4)
                    nc.sync.dma_start(out_idx[tok : tok + 128, :], idx4)

    nc.finalize()
    return nc


_PROGRAM_CACHE = {}


def _get_program():
    if "nc" not in _PROGRAM_CACHE:
        _PROGRAM_CACHE["nc"] = _build_program()
    return _PROGRAM_CACHE["nc"]


def kernel(hidden_states, weight, bias, _trace=False, _trace_kwargs=None):
    x = np.asarray(hidden_states, dtype=np.float32)
    w = np.asarray(weight, dtype=np.float32)
    b = np.asarray(bias, dtype=np.float32)
    assert x.shape == (T, H) and w.shape == (E, H) and b.shape == (E,)

    wTp = np.empty((HP, E), dtype=np.float32)
    wTp[:H] = w.T
    wTp[H] = b

    in_maps = []
    for i in range(N_CORES):
        shard = x[i * TC : (i + 1) * TC, :]  # [TC, H]
        # [HC*128, TC] -> [HC, 128, N_GROUPS, TG] -> [N_GROUPS, HC, 128, TG]
        xmain = np.ascontiguousarray(
            shard.T[: HC * 128]
            .reshape(HC, 128, N_GROUPS, TG)
            .transpose(2, 0, 1, 3)
        )
        xrem = np.empty((N_GROUPS, HREM, TG), dtype=np.float32)
        xrem[:, : H - HC * 128, :] = (
            shard.T[HC * 128 :].reshape(H - HC * 128, N_GROUPS, TG).transpose(1, 0, 2)
        )
        xrem[:, H - HC * 128 :, :] = 1.0  # ones row (bias)
        in_maps.append({"xm": xmain, "xr": xrem, "wT": wTp})

    nc = _get_program()
    kw = {}
    if _trace:
        kw = dict(trace=True, **(_trace_kwargs or {}))
    br = run_bass_kernel_spmd(nc, in_maps, list(range(N_CORES)), **kw)
    results = br.results

    vals = np.concatenate([results[i]["out_vals"] for i in range(N_CORES)], axis=0)
    idx = np.concatenate([results[i]["out_idx"] for i in range(N_CORES)], axis=0)
    logits = np.concatenate(
        [results[i]["out_logits"] for i in range(N_CORES)], axis=0
    )
    if _trace:
        return (vals, idx.astype(np.int32), logits), br
    return (vals, idx.astype(np.int32), logits)


# revision 14
# speedup vs baseline: 1.2195x; 1.2195x over previous
"""MoE router kernel (GPT-OSS TopK router) for 8 Trainium2 NeuronCores.

Computation (per reference):
    logits = hidden_states @ weight.T + bias        # [T=16384, E=128]
    top_v, top_i = top_k(logits, 4)                 # [T, 4]
    top_v = softmax(top_v, axis=-1)
    return (top_v, top_i, logits)

Sharding: token dim split 8 ways (2048 tokens/core); weight+bias replicated.
The x shard and weight are fed to the device pre-transposed and pre-tiled
into the exact per-group chunk order the kernel consumes, so every DMA
reads DRAM sequentially with 2KB descriptors. The bias is folded in by
appending a ones-row to xT and a bias-row to wT (H -> 2881 rows), so
logits = xT_pad.T @ wT_pad exactly (fp32 — float32r flips top-k indices).

Device pipeline per core, per 512-token group:
  4 sub-DMAs load the group's x chunks; 23-chunk fp32 matmul accumulation
  with the WEIGHT stationary into PSUM logitsT [128 exp, 512 tok]; ACT
  copy to SBUF; PE-transpose each 128-token tile back to [tok, exp]; ACT
  copy; DMA logits out; DVE max8/find_index8 for top-4 values+indices;
  ACT exp (bias=-max, accumulated sum); DVE reciprocal + scale. Scalar
  outputs are staged in SBUF and written once at the end.
"""

import sys

for _p in ("/opt/trn_rl_repo",):
    if _p not in sys.path:
        sys.path.insert(0, _p)

import numpy as np

import concourse.bass as bass
import concourse.mybir as mybir
from concourse import bacc
from concourse.bass_utils import run_bass_kernel_spmd
from concourse.masks import make_identity
from concourse.tile import TileContext

# Problem shape (hardcoded per contract)
T = 16384
H = 2880
E = 128
K = 4
N_CORES = 8
TC = T // N_CORES  # tokens per core = 2048

HP = H + 1  # padded contraction dim: ones/bias row folded in
HC = H // 128  # full 128-row contraction chunks = 22
HREM = HP - HC * 128  # 65 (64 x-rows + ones row)
# chunk ranges per x sub-DMA (earlier matmul start + finer DMA pipelining)
XSPLITS = [(0, 6), (6, 12), (12, 17), (17, 22)]

TG = 512  # tokens per group (PSUM bank free-dim)
N_GROUPS = TC // TG  # 4
SUBT = TG // 128  # 128-token subtiles per group = 4
N_TILES = TC // 128  # 16 token tiles per core

FP32 = mybir.dt.float32
I32 = mybir.dt.int32
U32 = mybir.dt.uint32


def _build_program():
    nc = bacc.Bacc("TRN2", target_bir_lowering=False, debug=False)

    # x main chunks, pre-tiled on host: [group, chunk, partition, token]
    xm = nc.declare_dram_parameter(
        "xm", [N_GROUPS, HC, 128, TG], FP32, isOutput=False
    ).ap()
    # x remainder rows (incl. ones row): [group, row, token]
    xr_d = nc.declare_dram_parameter(
        "xr", [N_GROUPS, HREM, TG], FP32, isOutput=False
    ).ap()
    wT = nc.declare_dram_parameter("wT", [HP, E], FP32, isOutput=False).ap()
    out_logits = nc.declare_dram_parameter(
        "out_logits", [TC, E], FP32, isOutput=True
    ).ap()
    out_vals = nc.declare_dram_parameter("out_vals", [TC, K], FP32, isOutput=True).ap()
    out_idx = nc.declare_dram_parameter("out_idx", [TC, K], I32, isOutput=True).ap()

    with TileContext(nc) as tc:
        with (
            tc.tile_pool(name="wpool", bufs=1) as wpool,
            tc.tile_pool(name="xpool", bufs=2) as xpool,
            tc.tile_pool(name="xrem", bufs=2) as xrempool,
            tc.tile_pool(name="psg", bufs=2, space="PSUM") as psg,
            tc.tile_pool(name="pst", bufs=4, space="PSUM") as pst,
            tc.tile_pool(name="ltpool", bufs=2) as ltpool,
            tc.tile_pool(name="lpool", bufs=3) as lpool,
            tc.tile_pool(name="small", bufs=4) as small,
        ):
            # One-time loads: transposed weight(+bias row), identity.
            # W is split so the first matmul only waits on chunks 0-5.
            WSPLIT = 6
            wt_main = wpool.tile([128, HC, E], FP32)
            nc.sync.dma_start(
                wt_main[:, :WSPLIT, :],
                wT[: WSPLIT * 128, :].rearrange("(c p) e -> p c e", p=128),
            )
            nc.sync.dma_start(
                wt_main[:, WSPLIT:, :],
                wT[WSPLIT * 128 : HC * 128, :].rearrange("(c p) e -> p c e", p=128),
            )
            wt_rem = wpool.tile([HREM, E], FP32)
            nc.sync.dma_start(wt_rem, wT[HC * 128 :, :])
            identity = wpool.tile([128, 128], FP32)
            make_identity(nc, identity)

            for g in range(N_GROUPS):
                xs = []
                for si, (c0, c1) in enumerate(XSPLITS):
                    xt = xpool.tile([128, c1 - c0, TG], FP32, tag=f"xs{si}")
                    nc.sync.dma_start(xt, xm[g, c0:c1].rearrange("c p t -> p c t"))
                    xs.append(xt)
                xr = xrempool.tile([HREM, TG], FP32)
                nc.sync.dma_start(xr, xr_d[g])

                # logitsT[e, t] accumulated over 23 chunks; weight stationary
                ps = psg.tile([128, TG], FP32)
                for si, (c0, c1) in enumerate(XSPLITS):
                    for c in range(c0, c1):
                        nc.tensor.matmul(
                            ps,
                            lhsT=wt_main[:, c, :],
                            rhs=xs[si][:, c - c0, :],
                            start=(c == 0),
                            stop=False,
                        )
                nc.tensor.matmul(ps, lhsT=wt_rem, rhs=xr, start=False, stop=True)

                ltT = ltpool.tile([128, TG], FP32)
                nc.scalar.copy(ltT, ps)

                for s in range(SUBT):
                    j = g * SUBT + s  # token tile index (0..15)
                    tok = j * 128

                    ps_t = pst.tile([128, 128], FP32)
                    nc.tensor.transpose(
                        ps_t, ltT[:, s * 128 : (s + 1) * 128], identity
                    )
                    logits_sb = lpool.tile([128, E], FP32)
                    nc.scalar.copy(logits_sb, ps_t)
                    nc.gpsimd.dma_start(out_logits[tok : tok + 128, :], logits_sb)

                    top8v = small.tile([128, 8], FP32)
                    nc.vector.max(top8v, logits_sb)
                    top8i = small.tile([128, 8], U32)
                    nc.vector.max_index(top8i, top8v, logits_sb)

                    negmax = small.tile([128, 1], FP32)
                    nc.vector.tensor_scalar_mul(negmax, top8v[:, 0:1], -1.0)
                    expv = small.tile([128, K], FP32)
                    sum4 = small.tile([128, 1], FP32)
                    nc.scalar.activation(
                        expv,
                        top8v[:, 0:K],
                        mybir.ActivationFunctionType.Exp,
                        bias=negmax,
                        scale=1.0,
                        accum_out=sum4,
                    )
                    rsum = small.tile([128, 1], FP32)
                    nc.vector.reciprocal(rsum, sum4)
                    vals4 = small.tile([128, K], FP32)
                    nc.vector.tensor_scalar_mul(vals4, expv, rsum)
                    idx4 = small.tile([128, K], I32)
                    nc.vector.tensor_copy(idx4, top8i[:, 0:K])
                    nc.gpsimd.dma_start(out_vals[tok : tok + 128, :], vals4)
                    nc.gpsimd.dma_start(out_idx[tok : tok + 128, :], idx4)

    nc.finalize()
    return nc


_PROGRAM_CACHE = {}


def _get_program():
    if "nc" not in _PROGRAM_CACHE:
        _PROGRAM_CACHE["nc"] = _build_program()
    return _PROGRAM_CACHE["nc"]


def kernel(hidden_states, weight, bias, _trace=False, _trace_kwargs=None):
    x = np.asarray(hidden_states, dtype=np.float32)
    w = np.asarray(weight, dtype=np.float32)
    b = np.asarray(bias, dtype=np.float32)
    assert x.shape == (T, H) and w.shape == (E, H) and b.shape == (E,)

    wTp = np.empty((HP, E), dtype=np.float32)
    wTp[:H] = w.T
    wTp[H] = b

    in_maps = []
    for i in range(N_CORES):
        shard = x[i * TC : (i + 1) * TC, :]  # [TC, H]
        # [HC*128, TC] -> [HC, 128, N_GROUPS, TG] -> [N_GROUPS, HC, 128, TG]
        xmain = np.ascontiguousarray(
            shard.T[: HC * 128]
            .reshape(HC, 128, N_GROUPS, TG)
            .transpose(2, 0, 1, 3)
        )
        xrem = np.empty((N_GROUPS, HREM, TG), dtype=np.float32)
        xrem[:, : H - HC * 128, :] = (
            shard.T[HC * 128 :].reshape(H - HC * 128, N_GROUPS, TG).transpose(1, 0, 2)
        )
        xrem[:, H - HC * 128 :, :] = 1.0  # ones row (bias)
        in_maps.append({"xm": xmain, "xr": xrem, "wT": wTp})

    nc = _get_program()
    kw = {}
    if _trace:
        kw = dict(trace=True, **(_trace_kwargs or {}))
    br = run_bass_kernel_spmd(nc, in_maps, list(range(N_CORES)), **kw)
    results = br.results

    vals = np.concatenate([results[i]["out_vals"] for i in range(N_CORES)], axis=0)
    idx = np.concatenate([results[i]["out_idx"] for i in range(N_CORES)], axis=0)
    logits = np.concatenate(
        [results[i]["out_logits"] for i in range(N_CORES)], axis=0
    )
    if _trace:
        return (vals, idx.astype(np.int32), logits), br
    return (vals, idx.astype(np.int32), logits)


# revision 17
# speedup vs baseline: 1.2830x; 1.0521x over previous
"""MoE router kernel (GPT-OSS TopK router) for 8 Trainium2 NeuronCores.

Computation (per reference):
    logits = hidden_states @ weight.T + bias        # [T=16384, E=128]
    top_v, top_i = top_k(logits, 4)                 # [T, 4]
    top_v = softmax(top_v, axis=-1)
    return (top_v, top_i, logits)

Sharding: token dim split 8 ways (2048 tokens/core); weight+bias replicated.
The x shard and weight are fed to the device pre-transposed and pre-tiled
into the exact per-group chunk order the kernel consumes, so every DMA
reads DRAM sequentially with 2KB descriptors. The bias is folded in by
appending a ones-row to xT and a bias-row to wT (H -> 2881 rows), so
logits = xT_pad.T @ wT_pad exactly (fp32 — float32r flips top-k indices).

Device pipeline per core, per 512-token group:
  4 sub-DMAs load the group's x chunks; 23-chunk fp32 matmul accumulation
  with the WEIGHT stationary into PSUM logitsT [128 exp, 512 tok]; ACT
  copy to SBUF; PE-transpose each 128-token tile back to [tok, exp]; ACT
  copy; DMA logits out; DVE max8/find_index8 for top-4 values+indices;
  ACT exp (bias=-max, accumulated sum); DVE reciprocal + scale. Scalar
  outputs are staged in SBUF and written once at the end.
"""

import sys

for _p in ("/opt/trn_rl_repo",):
    if _p not in sys.path:
        sys.path.insert(0, _p)

import numpy as np

import concourse.bass as bass
import concourse.mybir as mybir
from concourse import bacc
from concourse.bass_utils import run_bass_kernel_spmd
from concourse.masks import make_identity
from concourse.tile import TileContext

# Problem shape (hardcoded per contract)
T = 16384
H = 2880
E = 128
K = 4
N_CORES = 8
TC = T // N_CORES  # tokens per core = 2048

HP = H + 1  # padded contraction dim: ones/bias row folded in
HC = H // 128  # full 128-row contraction chunks = 22
HREM = HP - HC * 128  # 65 (64 x-rows + ones row)
# chunk ranges per x sub-DMA (earlier matmul start + finer DMA pipelining)
XSPLITS = [(0, 3), (3, 9), (9, 15), (15, 22)]

TG = 512  # tokens per group (PSUM bank free-dim)
N_GROUPS = TC // TG  # 4
SUBT = TG // 128  # 128-token subtiles per group = 4
N_TILES = TC // 128  # 16 token tiles per core

FP32 = mybir.dt.float32
I32 = mybir.dt.int32
U32 = mybir.dt.uint32


def _build_program():
    nc = bacc.Bacc("TRN2", target_bir_lowering=False, debug=False)

    # x main chunks, pre-tiled on host: [group, chunk, partition, token]
    xm = nc.declare_dram_parameter(
        "xm", [N_GROUPS, HC, 128, TG], FP32, isOutput=False
    ).ap()
    # x remainder rows (incl. ones row): [group, row, token]
    xr_d = nc.declare_dram_parameter(
        "xr", [N_GROUPS, HREM, TG], FP32, isOutput=False
    ).ap()
    wT = nc.declare_dram_parameter("wT", [HP, E], FP32, isOutput=False).ap()
    out_logits = nc.declare_dram_parameter(
        "out_logits", [TC, E], FP32, isOutput=True
    ).ap()
    out_vals = nc.declare_dram_parameter("out_vals", [TC, K], FP32, isOutput=True).ap()
    out_idx = nc.declare_dram_parameter("out_idx", [TC, K], I32, isOutput=True).ap()

    with TileContext(nc) as tc:
        with (
            tc.tile_pool(name="wpool", bufs=1) as wpool,
            tc.tile_pool(name="xpool", bufs=2) as xpool,
            tc.tile_pool(name="xrem", bufs=2) as xrempool,
            tc.tile_pool(name="psg", bufs=2, space="PSUM") as psg,
            tc.tile_pool(name="pst", bufs=4, space="PSUM") as pst,
            tc.tile_pool(name="ltpool", bufs=2) as ltpool,
            tc.tile_pool(name="lpool", bufs=3) as lpool,
            tc.tile_pool(name="small", bufs=4) as small,
        ):
            # One-time loads: transposed weight(+bias row), identity.
            # W parts are interleaved with group 0's x sub-loads so the
            # first matmuls' dependencies arrive as early as possible.
            WSPLITS = [(0, 3), (3, 9), (9, HC)]
            wt_main = wpool.tile([128, HC, E], FP32)
            wt_rem = wpool.tile([HREM, E], FP32)
            identity = wpool.tile([128, 128], FP32)
            make_identity(nc, identity)

            def load_w_part(wi):
                c0, c1 = WSPLITS[wi]
                nc.sync.dma_start(
                    wt_main[:, c0:c1, :],
                    wT[c0 * 128 : c1 * 128, :].rearrange("(c p) e -> p c e", p=128),
                )

            def load_x_part(g, si, xs):
                c0, c1 = XSPLITS[si]
                xt = xpool.tile([128, c1 - c0, TG], FP32, tag=f"xs{si}")
                nc.sync.dma_start(xt, xm[g, c0:c1].rearrange("c p t -> p c t"))
                xs.append(xt)

            for g in range(N_GROUPS):
                xs = []
                if g == 0:
                    load_w_part(0)
                    load_x_part(0, 0, xs)
                    load_w_part(1)
                    load_x_part(0, 1, xs)
                    load_w_part(2)
                    load_x_part(0, 2, xs)
                    load_x_part(0, 3, xs)
                    nc.sync.dma_start(wt_rem, wT[HC * 128 :, :])
                else:
                    for si in range(len(XSPLITS)):
                        load_x_part(g, si, xs)
                xr = xrempool.tile([HREM, TG], FP32)
                nc.sync.dma_start(xr, xr_d[g])

                # logitsT[e, t] accumulated over 23 chunks; weight stationary
                ps = psg.tile([128, TG], FP32)
                for si, (c0, c1) in enumerate(XSPLITS):
                    for c in range(c0, c1):
                        nc.tensor.matmul(
                            ps,
                            lhsT=wt_main[:, c, :],
                            rhs=xs[si][:, c - c0, :],
                            start=(c == 0),
                            stop=False,
                        )
                nc.tensor.matmul(ps, lhsT=wt_rem, rhs=xr, start=False, stop=True)

                ltT = ltpool.tile([128, TG], FP32)
                nc.scalar.copy(ltT, ps)

                for s in range(SUBT):
                    j = g * SUBT + s  # token tile index (0..15)
                    tok = j * 128

                    ps_t = pst.tile([128, 128], FP32)
                    nc.tensor.transpose(
                        ps_t, ltT[:, s * 128 : (s + 1) * 128], identity
                    )
                    logits_sb = lpool.tile([128, E], FP32)
                    nc.scalar.copy(logits_sb, ps_t)
                    nc.scalar.dma_start(out_logits[tok : tok + 128, :], logits_sb)

                    top8v = small.tile([128, 8], FP32)
                    nc.vector.max(top8v, logits_sb)
                    top8i = small.tile([128, 8], U32)
                    nc.vector.max_index(top8i, top8v, logits_sb)

                    negmax = small.tile([128, 1], FP32)
                    nc.vector.tensor_scalar_mul(negmax, top8v[:, 0:1], -1.0)
                    expv = small.tile([128, K], FP32)
                    sum4 = small.tile([128, 1], FP32)
                    nc.scalar.activation(
                        expv,
                        top8v[:, 0:K],
                        mybir.ActivationFunctionType.Exp,
                        bias=negmax,
                        scale=1.0,
                        accum_out=sum4,
                    )
                    rsum = small.tile([128, 1], FP32)
                    nc.vector.reciprocal(rsum, sum4)
                    vals4 = small.tile([128, K], FP32)
                    nc.vector.tensor_scalar_mul(vals4, expv, rsum)
                    idx4 = small.tile([128, K], I32)
                    nc.vector.tensor_copy(idx4, top8i[:, 0:K])
                    nc.gpsimd.dma_start(out_vals[tok : tok + 128, :], vals4)
                    nc.gpsimd.dma_start(out_idx[tok : tok + 128, :], idx4)

    nc.finalize()
    return nc


_PROGRAM_CACHE = {}


def _get_program():
    if "nc" not in _PROGRAM_CACHE:
        _PROGRAM_CACHE["nc"] = _build_program()
    return _PROGRAM_CACHE["nc"]


def kernel(hidden_states, weight, bias, _trace=False, _trace_kwargs=None):
    x = np.asarray(hidden_states, dtype=np.float32)
    w = np.asarray(weight, dtype=np.float32)
    b = np.asarray(bias, dtype=np.float32)
    assert x.shape == (T, H) and w.shape == (E, H) and b.shape == (E,)

    wTp = np.empty((HP, E), dtype=np.float32)
    wTp[:H] = w.T
    wTp[H] = b

    in_maps = []
    for i in range(N_CORES):
        shard = x[i * TC : (i + 1) * TC, :]  # [TC, H]
        # [HC*128, TC] -> [HC, 128, N_GROUPS, TG] -> [N_GROUPS, HC, 128, TG]
        xmain = np.ascontiguousarray(
            shard.T[: HC * 128]
            .reshape(HC, 128, N_GROUPS, TG)
            .transpose(2, 0, 1, 3)
        )
        xrem = np.empty((N_GROUPS, HREM, TG), dtype=np.float32)
        xrem[:, : H - HC * 128, :] = (
            shard.T[HC * 128 :].reshape(H - HC * 128, N_GROUPS, TG).transpose(1, 0, 2)
        )
        xrem[:, H - HC * 128 :, :] = 1.0  # ones row (bias)
        in_maps.append({"xm": xmain, "xr": xrem, "wT": wTp})

    nc = _get_program()
    kw = {}
    if _trace:
        kw = dict(trace=True, **(_trace_kwargs or {}))
    br = run_bass_kernel_spmd(nc, in_maps, list(range(N_CORES)), **kw)
    results = br.results

    vals = np.concatenate([results[i]["out_vals"] for i in range(N_CORES)], axis=0)
    idx = np.concatenate([results[i]["out_idx"] for i in range(N_CORES)], axis=0)
    logits = np.concatenate(
        [results[i]["out_logits"] for i in range(N_CORES)], axis=0
    )
    if _trace:
        return (vals, idx.astype(np.int32), logits), br
    return (vals, idx.astype(np.int32), logits)
